# revision 8
# baseline (speedup 1.0000x reference)
"""GCN 3-layer (DGL GraphConv, norm='both', zero biases) on 8 Trainium2 cores.

Math: with Nh = diag(deg_in^-1/2) A diag(deg_out^-1/2), the reference is
  h3 = Nh(Nh(Nh X W1) W2) W3   (biases are zero per spec)
Node-mixing (Nh) and feature-mixing (W) commute, so h3 = Nh^3 (X (W1 W2 W3)).
The kernel computes Wc = W1 W2 W3 on device and runs three aggregation passes.

Sharding: graph-level data parallel, 2 of the 16 component graphs per core
(8192 nodes, 131072 edges per core). Host does integer index preprocessing
only (dense per-block adjacency counts in fp8, exact for small ints).

v2: each graph's 16MB fp8 block-adjacency is loaded into SBUF ONCE per
3-layer pass (instead of re-streamed per layer): HBM traffic drops from
96MB to 32MB per iteration. SBUF rotation: per graph, the first NPRE dst
tiles live in a dedicated prefetch region (filled during the OTHER graph's
layers 2-3), the remaining tiles stream through a shared region during
layer 1. Node features Y are plain bf16 (tolerance permits), so each
aggregation block matmul moves only 64 columns:
  psum[128d x 64f] += A_block[128s x 128d]^T . Y[128s x 64f]
"""

import os
import functools
import numpy as np

import concourse.bacc as bacc
import concourse.mybir as mybir
import concourse.tile as tile
from concourse.masks import make_identity

F32 = mybir.dt.float32
BF16 = mybir.dt.bfloat16
FP8 = mybir.dt.float8e4

NUM_NODES = 65536
NODES_PER_GRAPH = 4096
NUM_GRAPHS = 16
NUM_EDGES = 1048576
D = 64
NCORES = 8
NPC = NUM_NODES // NCORES          # 8192 nodes per core
EPC = NUM_EDGES // NCORES          # 131072 edges per core
NT = NPC // 128                    # 64 node tiles per core
TPG = NODES_PER_GRAPH // 128       # 32 node tiles per graph
NPRE = 14                          # dst tiles per graph in prefetch region
SQ_SPLIT = (4, 4, 4, 3, 3)         # streamed dst tiles per shared slot piece
assert sum(SQ_SPLIT) == TPG - NPRE
SIM = bool(int(os.environ.get("GCN_SIM", "0")))


# ----------------------------------------------------------------------------
# Host preprocessing (integer index work only)
# ----------------------------------------------------------------------------

def _preprocess(src, dst):
    """Per-core fp8 block-adjacency + degree arrays."""
    fp8 = mybir.dt.np(FP8)
    out = []
    for c in range(NCORES):
        e0, e1 = c * EPC, (c + 1) * EPC
        n0 = c * NPC
        s = src[e0:e1] - n0
        d = dst[e0:e1] - n0
        assert s.min() >= 0 and s.max() < NPC and d.min() >= 0 and d.max() < NPC
        deg_out = np.bincount(s, minlength=NPC).astype(np.float32)
        deg_in = np.bincount(d, minlength=NPC).astype(np.float32)
        # A[s%128, i, jj, d%128] = edge count  (i = dst tile, jj = src tile
        # local to its graph; graphs are edge-disjoint by construction)
        sp = s % 128
        jg = s // 128
        g = s // NODES_PER_GRAPH
        jj = jg - TPG * g
        i = d // 128
        dp = d % 128
        assert np.array_equal(i // TPG, g), "cross-graph edge"
        flat = ((sp * NT + i) * TPG + jj) * 128 + dp
        counts = np.bincount(flat, minlength=128 * NT * TPG * 128)
        counts = counts.reshape(128, NT * TPG * 128).astype(np.float32)
        A = counts.astype(fp8)
        assert np.array_equal(A.astype(np.float32), counts), "fp8 inexact count"
        out.append(dict(
            A=A,
            deg_out=np.ascontiguousarray(deg_out.reshape(NT, 128).T),
            deg_in=np.ascontiguousarray(deg_in.reshape(NT, 128).T),
        ))
    return out


# ----------------------------------------------------------------------------
# Device program
# ----------------------------------------------------------------------------

def _normify(nc, pool, deg, shape, tag):
    """norm = (deg>0) * 1/sqrt(max(deg,1)) ; matches the reference formula."""
    t = pool.tile(shape, F32, tag=f"{tag}_tmp")
    r = pool.tile(shape, F32, tag=f"{tag}_r")
    m = pool.tile(shape, F32, tag=f"{tag}_m")
    o = pool.tile(shape, F32, tag=f"{tag}_o")
    nc.vector.tensor_scalar_max(t[:], deg[:], 1.0)
    nc.vector.reciprocal(r[:], t[:])
    nc.scalar.activation(r[:], r[:], mybir.ActivationFunctionType.Sqrt)
    nc.vector.tensor_scalar(m[:], deg[:], 0.0, None, mybir.AluOpType.is_gt)
    nc.vector.tensor_mul(o[:], r[:], m[:])
    return o


def build_program(reps=1, grp=None):
    nc = bacc.Bacc(None)
    GRP = grp or int(os.environ.get("GCN_GRP", "4"))  # dst tiles per psum group

    xT = nc.dram_tensor("xT", [D, NPC], F32, kind="ExternalInput")
    W1 = nc.dram_tensor("W1", [D, D], F32, kind="ExternalInput")
    W2 = nc.dram_tensor("W2", [D, D], F32, kind="ExternalInput")
    W3 = nc.dram_tensor("W3", [D, D], F32, kind="ExternalInput")
    A_in = nc.dram_tensor("A", [128, NT * TPG * 128], FP8, kind="ExternalInput")
    dego = nc.dram_tensor("deg_out", [128, NT], F32, kind="ExternalInput")
    degi = nc.dram_tensor("deg_in", [128, NT], F32, kind="ExternalInput")
    out = nc.dram_tensor("out", [NPC, D], F32, kind="ExternalOutput")

    def a_cols(t0, t1):
        """A_in column range for global dst tiles [t0, t1)."""
        return A_in[:, t0 * TPG * 128:t1 * TPG * 128].rearrange(
            "s (t j d) -> s t j d", j=TPG, d=128)

    with tile.TileContext(nc) as tc:
        with tc.tile_pool(name="persist", bufs=1) as pp:
            do = pp.tile([128, NT], F32)
            di = pp.tile([128, NT], F32)
            nc.sync.dma_start(do[:], dego[:])
            nc.sync.dma_start(di[:], degi[:])
            ns = _normify(nc, pp, do, [128, NT], "n1")
            nd = _normify(nc, pp, di, [128, NT], "n2")
            cs = pp.tile([128, NT], F32)
            nc.vector.tensor_mul(cs[:], ns[:], nd[:])

            # node features: Y0 (all 64 tiles) + two per-graph work buffers
            Y0 = pp.tile([128, NT, D], BF16, name="Y0")
            Yw = [pp.tile([128, TPG, D], BF16, name=f"Yw{k}") for k in range(2)]

            # --- Wc = W1 @ W2 @ W3 ; Y0 = bf16(ns * (X @ Wc)) ---
            with (
                tc.tile_pool(name="winit", bufs=1) as wp,
                tc.tile_pool(name="winit_ps", bufs=1, space="PSUM") as wps,
            ):
                ident = wp.tile([128, 128], F32)
                make_identity(nc, ident[:])
                w1 = wp.tile([D, D], F32)
                w2 = wp.tile([D, D], F32)
                w3 = wp.tile([D, D], F32)
                nc.sync.dma_start(w1[:], W1[:])
                nc.sync.dma_start(w2[:], W2[:])
                nc.sync.dma_start(w3[:], W3[:])
                ps = wps.tile([D, D], F32, tag="wps")
                w1t = wp.tile([D, D], F32)
                nc.tensor.transpose(ps[:], w1[:], ident[:D, :D])
                nc.vector.tensor_copy(w1t[:], ps[:])
                ps12 = wps.tile([D, D], F32, tag="wps12")
                w12 = wp.tile([D, D], F32)
                nc.tensor.matmul(ps12[:], w1t[:], w2[:], start=True, stop=True)
                nc.vector.tensor_copy(w12[:], ps12[:])
                ps12t = wps.tile([D, D], F32, tag="wps12t")
                w12t = wp.tile([D, D], F32)
                nc.tensor.transpose(ps12t[:], w12[:], ident[:D, :D])
                nc.vector.tensor_copy(w12t[:], ps12t[:])
                psc = wps.tile([D, D], F32, tag="wpsc")
                wc = pp.tile([D, D], F32)
                nc.tensor.matmul(psc[:], w12t[:], w3[:], start=True, stop=True)
                nc.vector.tensor_copy(wc[:], psc[:])

                xt_sb = wp.tile([D, NPC], F32)
                nc.sync.dma_start(xt_sb[:], xT[:])
                for j in range(NT):
                    zps = wps.tile([128, D], F32, tag="z0ps")
                    nc.tensor.matmul(
                        zps[:], xt_sb[:, j * 128:(j + 1) * 128], wc[:],
                        start=True, stop=True,
                    )
                    t32 = wp.tile([128, D], F32, tag="z0t32")
                    nc.vector.tensor_mul(
                        t32[:], zps[:], ns[:, j:j + 1].to_broadcast([128, D]))
                    nc.vector.tensor_copy(Y0[:, j, :], t32[:])

            # --- 3 aggregation layers per graph, A resident across layers ---
            with (
                tc.tile_pool(name="lay", bufs=1) as lp,
                tc.tile_pool(name="lay_ps", bufs=2, space="PSUM") as lps,
                tc.tile_pool(name="epi", bufs=2) as ep,
            ):
                # A slots: per-graph prefetch regions + shared streamed region
                P = [lp.tile([128, NPRE, TPG, 128], FP8, name=f"APre{g}")
                     for g in range(2)]
                SQ = [lp.tile([128, n, TPG, 128], FP8, name=f"ASq{k}")
                      for k, n in enumerate(SQ_SPLIT)]
                sq_off = np.cumsum((0,) + SQ_SPLIT)

                def a_blk(g, t, jj):
                    """Stationary [128s,128d] for graph g, local dst tile t."""
                    if t < NPRE:
                        return P[g][:, t, jj, :]
                    for k in range(len(SQ_SPLIT)):
                        if t - NPRE < sq_off[k + 1]:
                            return SQ[k][:, t - NPRE - sq_off[k], jj, :]
                    raise AssertionError

                # prologue: graph 0 prefetch region
                nc.sync.dma_start(P[0][:], a_cols(0, NPRE))

                import contextlib
                loop_ctx = (tc.For_i(0, reps, 1) if reps > 1
                            else contextlib.nullcontext())
                with loop_ctx:
                  for g in range(2):
                    gt = g * TPG
                    # stream this graph's tail tiles into the shared region
                    for k in range(len(SQ_SPLIT)):
                        nc.sync.dma_start(
                            SQ[k][:],
                            a_cols(gt + NPRE + sq_off[k],
                                   gt + NPRE + sq_off[k + 1]))
                    # refill the other prefetch region (next graph / next rep)
                    og = 1 - g
                    nc.sync.dma_start(
                        P[og][:], a_cols(og * TPG, og * TPG + NPRE))

                    ngrp = TPG // GRP
                    # groups touching the shared SQ region, then pure-P groups
                    sq_first = ([i for i in range(ngrp) if (i + 1) * GRP > NPRE]
                                + [i for i in range(ngrp) if (i + 1) * GRP <= NPRE])
                    for layer in range(3):
                        last = layer == 2
                        y_src = Y0 if layer == 0 else Yw[layer - 1]
                        y_dst = Yw[layer] if not last else None
                        scale = nd if last else cs
                        # L3 releases SQ early so the next graph's stream can
                        # refill it under the remainder of this layer
                        for ig in (sq_first if last else range(ngrp)):
                            psq = [lps.tile([128, D], F32, tag=f"aggps{q}",
                                            name=f"ps_{g}_{layer}_{ig}_{q}")
                                   for q in range(GRP)]
                            for jj in range(TPG):
                                yj = (Y0[:, gt + jj, :] if layer == 0
                                      else y_src[:, jj, :])
                                for q in range(GRP):
                                    nc.tensor.matmul(
                                        psq[q][:],
                                        a_blk(g, ig * GRP + q, jj),
                                        yj,
                                        start=(jj == 0), stop=(jj == TPG - 1),
                                    )
                            i0 = ig * GRP
                            if not last:
                                for q in range(GRP):
                                    sc_q = scale[:, gt + i0 + q:gt + i0 + q + 1]
                                    nc.scalar.activation(
                                        y_dst[:, i0 + q, :], psq[q][:],
                                        mybir.ActivationFunctionType.Copy,
                                        scale=sc_q)
                            else:
                                o32 = ep.tile([128, GRP, D], F32, tag="o32")
                                for q in range(GRP):
                                    sc_q = scale[:, gt + i0 + q:gt + i0 + q + 1]
                                    nc.scalar.activation(
                                        o32[:, q, :], psq[q][:],
                                        mybir.ActivationFunctionType.Copy,
                                        scale=sc_q)
                                n0 = (gt + i0) * 128
                                nc.sync.dma_start(
                                    out[n0:n0 + GRP * 128, :].rearrange(
                                        "(c p) f -> p c f", p=128),
                                    o32[:],
                                )
    nc.finalize()
    return nc


@functools.lru_cache(maxsize=2)
def _cached_program():
    return build_program(reps=int(os.environ.get("GCN_REPS", "1")))


# ----------------------------------------------------------------------------
# Entry point
# ----------------------------------------------------------------------------

def make_in_maps(x, W1, W2, W3, per_core):
    in_maps = []
    for c in range(NCORES):
        pc = per_core[c]
        xs = x[c * NPC:(c + 1) * NPC]
        in_maps.append({
            "xT": np.ascontiguousarray(xs.T),
            "W1": W1, "W2": W2, "W3": W3,
            "A": pc["A"],
            "deg_out": pc["deg_out"],
            "deg_in": pc["deg_in"],
        })
    return in_maps


def kernel(x, W1, b1, W2, b2, W3, b3, src, dst, num_graphs):
    x = np.asarray(x, dtype=np.float32)
    W1 = np.asarray(W1, dtype=np.float32)
    W2 = np.asarray(W2, dtype=np.float32)
    W3 = np.asarray(W3, dtype=np.float32)
    for b in (b1, b2, b3):
        assert not np.any(np.asarray(b)), "nonzero biases unsupported"
    src = np.asarray(src, dtype=np.int64)
    dst = np.asarray(dst, dtype=np.int64)

    per_core = _preprocess(src, dst)
    nc = _cached_program()
    in_maps = make_in_maps(x, W1, W2, W3, per_core)

    if SIM:
        from concourse import bass_interp
        sim = bass_interp.CoreSim(nc)
        for name, arr in in_maps[0].items():
            sim.tensor(name)[:] = arr
        sim.simulate()
        print(f"[sim] core0 estimated time: {sim.time} ns")
        o = np.array(sim.tensor("out"))
        res = np.concatenate([o] + [np.zeros_like(o)] * (NCORES - 1))
        return res.reshape(NUM_GRAPHS, NODES_PER_GRAPH, D)

    from concourse.bass_utils import run_bass_kernel_spmd
    res = run_bass_kernel_spmd(nc, in_maps, core_ids=list(range(NCORES)))
    full = np.concatenate([res.results[c]["out"] for c in range(NCORES)], axis=0)
    return full.reshape(NUM_GRAPHS, NODES_PER_GRAPH, D)


# revision 9
# speedup vs baseline: 1.1901x; 1.1901x over previous
"""GCN 3-layer (DGL GraphConv, norm='both', zero biases) on 8 Trainium2 cores.

Math: with Nh = diag(deg_in^-1/2) A diag(deg_out^-1/2), the reference is
  h3 = Nh(Nh(Nh X W1) W2) W3   (biases are zero per spec)
Node-mixing (Nh) and feature-mixing (W) commute, so h3 = Nh^3 (X (W1 W2 W3)).
The kernel computes Wc = W1 W2 W3 on device and runs three aggregation passes.

Sharding: graph-level data parallel, 2 of the 16 component graphs per core
(8192 nodes, 131072 edges per core). Host does integer index preprocessing
only (dense per-block adjacency counts in fp8, exact for small ints).

v2: each graph's 16MB fp8 block-adjacency is loaded into SBUF ONCE per
3-layer pass (instead of re-streamed per layer): HBM traffic drops from
96MB to 32MB per iteration. SBUF rotation: per graph, the first NPRE dst
tiles live in a dedicated prefetch region (filled during the OTHER graph's
layers 2-3), the remaining tiles stream through a shared region during
layer 1. Node features Y are plain bf16 (tolerance permits), so each
aggregation block matmul moves only 64 columns:
  psum[128d x 64f] += A_block[128s x 128d]^T . Y[128s x 64f]
"""

import os
import functools
import numpy as np

import concourse.bacc as bacc
import concourse.mybir as mybir
import concourse.tile as tile
from concourse.masks import make_identity

F32 = mybir.dt.float32
BF16 = mybir.dt.bfloat16
FP8 = mybir.dt.float8e4

NUM_NODES = 65536
NODES_PER_GRAPH = 4096
NUM_GRAPHS = 16
NUM_EDGES = 1048576
D = 64
NCORES = 8
NPC = NUM_NODES // NCORES          # 8192 nodes per core
EPC = NUM_EDGES // NCORES          # 131072 edges per core
NT = NPC // 128                    # 64 node tiles per core
TPG = NODES_PER_GRAPH // 128       # 32 node tiles per graph
NPRE = 13                          # dst tiles per graph in prefetch region
SQ_SPLIT = (5, 5, 5, 4)            # streamed dst tiles per shared slot piece
assert sum(SQ_SPLIT) == TPG - NPRE
SIM = bool(int(os.environ.get("GCN_SIM", "0")))


# ----------------------------------------------------------------------------
# Host preprocessing (integer index work only)
# ----------------------------------------------------------------------------

def _preprocess(src, dst):
    """Per-core fp8 block-adjacency + degree arrays."""
    fp8 = mybir.dt.np(FP8)
    out = []
    for c in range(NCORES):
        e0, e1 = c * EPC, (c + 1) * EPC
        n0 = c * NPC
        s = src[e0:e1] - n0
        d = dst[e0:e1] - n0
        assert s.min() >= 0 and s.max() < NPC and d.min() >= 0 and d.max() < NPC
        deg_out = np.bincount(s, minlength=NPC).astype(np.float32)
        deg_in = np.bincount(d, minlength=NPC).astype(np.float32)
        # A[s%128, i, jj, d%128] = edge count  (i = dst tile, jj = src tile
        # local to its graph; graphs are edge-disjoint by construction)
        sp = s % 128
        jg = s // 128
        g = s // NODES_PER_GRAPH
        jj = jg - TPG * g
        i = d // 128
        dp = d % 128
        assert np.array_equal(i // TPG, g), "cross-graph edge"
        flat = ((sp * NT + i) * TPG + jj) * 128 + dp
        counts = np.bincount(flat, minlength=128 * NT * TPG * 128)
        counts = counts.reshape(128, NT * TPG * 128).astype(np.float32)
        A = counts.astype(fp8)
        assert np.array_equal(A.astype(np.float32), counts), "fp8 inexact count"
        out.append(dict(
            A=A,
            deg_out=np.ascontiguousarray(deg_out.reshape(NT, 128).T),
            deg_in=np.ascontiguousarray(deg_in.reshape(NT, 128).T),
        ))
    return out


# ----------------------------------------------------------------------------
# Device program
# ----------------------------------------------------------------------------

def _normify(nc, pool, deg, shape, tag):
    """norm = (deg>0) * 1/sqrt(max(deg,1)) ; matches the reference formula."""
    t = pool.tile(shape, F32, tag=f"{tag}_tmp")
    r = pool.tile(shape, F32, tag=f"{tag}_r")
    m = pool.tile(shape, F32, tag=f"{tag}_m")
    o = pool.tile(shape, F32, tag=f"{tag}_o")
    nc.vector.tensor_scalar_max(t[:], deg[:], 1.0)
    nc.vector.reciprocal(r[:], t[:])
    nc.scalar.activation(r[:], r[:], mybir.ActivationFunctionType.Sqrt)
    nc.vector.tensor_scalar(m[:], deg[:], 0.0, None, mybir.AluOpType.is_gt)
    nc.vector.tensor_mul(o[:], r[:], m[:])
    return o


def build_program(reps=1, grp=None):
    nc = bacc.Bacc(None)
    GRP = grp or int(os.environ.get("GCN_GRP", "4"))  # dst tiles per psum group

    xT = nc.dram_tensor("xT", [D, NPC], F32, kind="ExternalInput")
    W1 = nc.dram_tensor("W1", [D, D], F32, kind="ExternalInput")
    W2 = nc.dram_tensor("W2", [D, D], F32, kind="ExternalInput")
    W3 = nc.dram_tensor("W3", [D, D], F32, kind="ExternalInput")
    A_in = nc.dram_tensor("A", [128, NT * TPG * 128], FP8, kind="ExternalInput")
    dego = nc.dram_tensor("deg_out", [128, NT], F32, kind="ExternalInput")
    degi = nc.dram_tensor("deg_in", [128, NT], F32, kind="ExternalInput")
    out = nc.dram_tensor("out", [NPC, D], F32, kind="ExternalOutput")

    def a_cols(t0, t1):
        """A_in column range for global dst tiles [t0, t1)."""
        return A_in[:, t0 * TPG * 128:t1 * TPG * 128].rearrange(
            "s (t j d) -> s t j d", j=TPG, d=128)

    with tile.TileContext(nc) as tc:
        with tc.tile_pool(name="persist", bufs=1) as pp:
            do = pp.tile([128, NT], F32)
            di = pp.tile([128, NT], F32)
            nc.sync.dma_start(do[:], dego[:])
            nc.sync.dma_start(di[:], degi[:])
            ns = _normify(nc, pp, do, [128, NT], "n1")
            nd = _normify(nc, pp, di, [128, NT], "n2")
            cs = pp.tile([128, NT], F32)
            nc.vector.tensor_mul(cs[:], ns[:], nd[:])

            # node features: Y0 (all 64 tiles) + two per-graph work buffers
            Y0 = pp.tile([128, NT, D], BF16, name="Y0")
            Yw = [pp.tile([128, TPG, D], BF16, name=f"Yw{k}") for k in range(2)]

            # --- Wc = W1 @ W2 @ W3 ; Y0 = bf16(ns * (X @ Wc)) ---
            with (
                tc.tile_pool(name="winit", bufs=1) as wp,
                tc.tile_pool(name="winit_ps", bufs=1, space="PSUM") as wps,
            ):
                ident = wp.tile([128, 128], F32)
                make_identity(nc, ident[:])
                w1 = wp.tile([D, D], F32)
                w2 = wp.tile([D, D], F32)
                w3 = wp.tile([D, D], F32)
                nc.sync.dma_start(w1[:], W1[:])
                nc.sync.dma_start(w2[:], W2[:])
                nc.sync.dma_start(w3[:], W3[:])
                ps = wps.tile([D, D], F32, tag="wps")
                w1t = wp.tile([D, D], F32)
                nc.tensor.transpose(ps[:], w1[:], ident[:D, :D])
                nc.vector.tensor_copy(w1t[:], ps[:])
                ps12 = wps.tile([D, D], F32, tag="wps12")
                w12 = wp.tile([D, D], F32)
                nc.tensor.matmul(ps12[:], w1t[:], w2[:], start=True, stop=True)
                nc.vector.tensor_copy(w12[:], ps12[:])
                ps12t = wps.tile([D, D], F32, tag="wps12t")
                w12t = wp.tile([D, D], F32)
                nc.tensor.transpose(ps12t[:], w12[:], ident[:D, :D])
                nc.vector.tensor_copy(w12t[:], ps12t[:])
                psc = wps.tile([D, D], F32, tag="wpsc")
                wc = pp.tile([D, D], F32)
                nc.tensor.matmul(psc[:], w12t[:], w3[:], start=True, stop=True)
                nc.vector.tensor_copy(wc[:], psc[:])

                xt_sb = wp.tile([D, NPC], F32)
                nc.sync.dma_start(xt_sb[:], xT[:])
                for j in range(NT):
                    zps = wps.tile([128, D], F32, tag="z0ps")
                    nc.tensor.matmul(
                        zps[:], xt_sb[:, j * 128:(j + 1) * 128], wc[:],
                        start=True, stop=True,
                    )
                    t32 = wp.tile([128, D], F32, tag="z0t32")
                    nc.vector.tensor_mul(
                        t32[:], zps[:], ns[:, j:j + 1].to_broadcast([128, D]))
                    nc.vector.tensor_copy(Y0[:, j, :], t32[:])

            # --- 3 aggregation layers per graph, A resident across layers ---
            with (
                tc.tile_pool(name="lay", bufs=1) as lp,
                tc.tile_pool(name="lay_ps", bufs=2, space="PSUM") as lps,
                tc.tile_pool(name="epi", bufs=2) as ep,
            ):
                # A slots: per-graph prefetch regions + shared streamed region
                P = [lp.tile([128, NPRE, TPG, 128], FP8, name=f"APre{g}")
                     for g in range(2)]
                SQ = [lp.tile([128, n, TPG, 128], FP8, name=f"ASq{k}")
                      for k, n in enumerate(SQ_SPLIT)]
                sq_off = np.cumsum((0,) + SQ_SPLIT)

                def a_blk(g, t, jj):
                    """Stationary [128s,128d] for graph g, local dst tile t."""
                    if t < NPRE:
                        return P[g][:, t, jj, :]
                    for k in range(len(SQ_SPLIT)):
                        if t - NPRE < sq_off[k + 1]:
                            return SQ[k][:, t - NPRE - sq_off[k], jj, :]
                    raise AssertionError

                # prologue: graph 0 prefetch region
                nc.sync.dma_start(P[0][:], a_cols(0, NPRE))

                import contextlib
                loop_ctx = (tc.For_i(0, reps, 1) if reps > 1
                            else contextlib.nullcontext())
                with loop_ctx:
                  for g in range(2):
                    gt = g * TPG
                    # stream this graph's tail tiles into the shared region
                    for k in range(len(SQ_SPLIT)):
                        nc.sync.dma_start(
                            SQ[k][:],
                            a_cols(gt + NPRE + sq_off[k],
                                   gt + NPRE + sq_off[k + 1]))
                    # refill the other prefetch region (next graph / next rep)
                    og = 1 - g
                    nc.sync.dma_start(
                        P[og][:], a_cols(og * TPG, og * TPG + NPRE))

                    ngrp = TPG // GRP
                    # groups touching the shared SQ region, then pure-P groups
                    sq_first = ([i for i in range(ngrp) if (i + 1) * GRP > NPRE]
                                + [i for i in range(ngrp) if (i + 1) * GRP <= NPRE])
                    for layer in range(3):
                        last = layer == 2
                        y_src = Y0 if layer == 0 else Yw[layer - 1]
                        y_dst = Yw[layer] if not last else None
                        scale = nd if last else cs
                        # L3 releases SQ early so the next graph's stream can
                        # refill it under the remainder of this layer
                        for ig in (sq_first if last else range(ngrp)):
                            psq = [lps.tile([128, D], F32, tag=f"aggps{q}",
                                            name=f"ps_{g}_{layer}_{ig}_{q}")
                                   for q in range(GRP)]
                            for jj in range(TPG):
                                yj = (Y0[:, gt + jj, :] if layer == 0
                                      else y_src[:, jj, :])
                                for q in range(GRP):
                                    nc.tensor.matmul(
                                        psq[q][:],
                                        a_blk(g, ig * GRP + q, jj),
                                        yj,
                                        start=(jj == 0), stop=(jj == TPG - 1),
                                    )
                            i0 = ig * GRP
                            if not last:
                                for q in range(GRP):
                                    sc_q = scale[:, gt + i0 + q:gt + i0 + q + 1]
                                    nc.scalar.activation(
                                        y_dst[:, i0 + q, :], psq[q][:],
                                        mybir.ActivationFunctionType.Copy,
                                        scale=sc_q)
                            else:
                                o32 = ep.tile([128, GRP, D], F32, tag="o32")
                                for q in range(GRP):
                                    sc_q = scale[:, gt + i0 + q:gt + i0 + q + 1]
                                    nc.scalar.activation(
                                        o32[:, q, :], psq[q][:],
                                        mybir.ActivationFunctionType.Copy,
                                        scale=sc_q)
                                n0 = (gt + i0) * 128
                                nc.sync.dma_start(
                                    out[n0:n0 + GRP * 128, :].rearrange(
                                        "(c p) f -> p c f", p=128),
                                    o32[:],
                                )
    nc.finalize()
    return nc


@functools.lru_cache(maxsize=2)
def _cached_program():
    return build_program(reps=int(os.environ.get("GCN_REPS", "1")))


# ----------------------------------------------------------------------------
# Entry point
# ----------------------------------------------------------------------------

def make_in_maps(x, W1, W2, W3, per_core):
    in_maps = []
    for c in range(NCORES):
        pc = per_core[c]
        xs = x[c * NPC:(c + 1) * NPC]
        in_maps.append({
            "xT": np.ascontiguousarray(xs.T),
            "W1": W1, "W2": W2, "W3": W3,
            "A": pc["A"],
            "deg_out": pc["deg_out"],
            "deg_in": pc["deg_in"],
        })
    return in_maps


def kernel(x, W1, b1, W2, b2, W3, b3, src, dst, num_graphs):
    x = np.asarray(x, dtype=np.float32)
    W1 = np.asarray(W1, dtype=np.float32)
    W2 = np.asarray(W2, dtype=np.float32)
    W3 = np.asarray(W3, dtype=np.float32)
    for b in (b1, b2, b3):
        assert not np.any(np.asarray(b)), "nonzero biases unsupported"
    src = np.asarray(src, dtype=np.int64)
    dst = np.asarray(dst, dtype=np.int64)

    per_core = _preprocess(src, dst)
    nc = _cached_program()
    in_maps = make_in_maps(x, W1, W2, W3, per_core)

    if SIM:
        from concourse import bass_interp
        sim = bass_interp.CoreSim(nc)
        for name, arr in in_maps[0].items():
            sim.tensor(name)[:] = arr
        sim.simulate()
        print(f"[sim] core0 estimated time: {sim.time} ns")
        o = np.array(sim.tensor("out"))
        res = np.concatenate([o] + [np.zeros_like(o)] * (NCORES - 1))
        return res.reshape(NUM_GRAPHS, NODES_PER_GRAPH, D)

    from concourse.bass_utils import run_bass_kernel_spmd
    res = run_bass_kernel_spmd(nc, in_maps, core_ids=list(range(NCORES)))
    full = np.concatenate([res.results[c]["out"] for c in range(NCORES)], axis=0)
    return full.reshape(NUM_GRAPHS, NODES_PER_GRAPH, D)
